# revision 4
# baseline (speedup 1.0000x reference)
"""BitLinear (ternary-quantized linear) Trainium2 kernel.

Computes y = x @ w_q^T where w_q = clip(round(w/(alpha+eps)), -1, 1) * alpha
and alpha = mean(|w|) over the FULL weight.

Distribution (8 NeuronCores, tensor-parallel):
  - weight rows (out_features) sharded 8 ways: each core owns N = 16384/8 = 2048
    output features.
  - x replicated to every core (pre-transposed + bf16 on host for layout).
  - alpha needs the global |w| mean: each core reduces its own shard, then an
    on-device AllReduce across the 8 cores produces the global sum.
  - outputs stay column-sharded; host concatenates the 8 shards.

Per-core device kernel:
  phase 1: abs-sum reduce of w shard (DVE reduce w/ apply_absolute_value),
           cross-partition sum via PE ones-matmul, AllReduce, then
           alpha = S * 2^-26, r = 1/(alpha+eps) (DVE reciprocal).
  phase 2: quantize w -> t in {-1,0,+1} as bf16 (exact), using the fp32
           round-to-nearest-even "magic number" trick: RNE(z) = (z+1.5*2^23)-1.5*2^23.
           t kept entirely in SBUF ([128, 32, 2048] bf16 = 128KB/partition).
  phase 3: y = x @ t^T on the PE in bf16 (fp32 PSUM accumulation), tiled
           [128m x 128k x 512n]; PSUM evicted with a fused *alpha scale on DVE.
"""

import numpy as np
import ml_dtypes

import concourse.bass as bass
import concourse.mybir as mybir
import concourse.tile as tile
from concourse import bacc
from concourse.bass_utils import run_bass_kernel_spmd

P = 128
N_CORES = 8

# Full problem shapes (hardcoded per contract).
B, S, K = 4, 2048, 4096
N_FULL = 16384
M = B * S                      # 8192 rows of x
N = N_FULL // N_CORES          # 2048 output features per core

MAGIC = 12582912.0             # 1.5 * 2**23: fp32 RNE rounding constant
EPS = 1e-8
MEAN_SCALE = float(2.0 ** -26)  # 1 / (16384*4096), exact power of two

F32 = mybir.dt.float32
BF16 = mybir.dt.bfloat16


def build_nc(M, K, N, n_cores=N_CORES, mc=256, qw=1024):
    """Build the per-core Bass program. All cores run the same program (SPMD)."""
    KO = K // P          # k tiles of 128 partitions
    NT = N // 512        # matmul n tiles (psum banks per m-subtile)
    NQ = max(N // qw, 1) # quantization column tiles per k tile
    qw = min(qw, N)
    MCH = M // mc        # m chunks
    MS = mc // P         # m subtiles per chunk

    nc = bacc.Bacc("TRN2", target_bir_lowering=False, debug=False,
                   num_devices=n_cores)

    xt = nc.dram_tensor("xt", [K, M], BF16, kind="ExternalInput")
    wt = nc.dram_tensor("wt", [K, N], F32, kind="ExternalInput")
    y = nc.dram_tensor("y", [M, N], F32, kind="ExternalOutput")
    cc_in = nc.dram_tensor("cc_in", [1, 1], F32)
    cc_out = nc.dram_tensor("cc_out", [1, 1], F32, addr_space="Shared")

    w_view = wt.ap().rearrange("(ko p) n -> ko p n", p=P)    # [KO, 128, N]
    xt_view = xt.ap().rearrange("(ko p) m -> p ko m", p=P)   # [128, KO, M]
    y_view = y.ap().rearrange("(mo p) n -> mo p n", p=P)     # [M/128, 128, N]

    with tile.TileContext(nc) as tc:
        with (
            tc.tile_pool(name="const", bufs=1) as const,
            tc.tile_pool(name="scal", bufs=1) as scal,
            tc.tile_pool(name="wstage", bufs=3) as wstage,
            tc.tile_pool(name="tmp", bufs=2) as tmpp,
            tc.tile_pool(name="tpool", bufs=1) as tpool,
            tc.tile_pool(name="xstage", bufs=2) as xstage,
            tc.tile_pool(name="outp", bufs=2) as outp,
            tc.tile_pool(name="psum", bufs=1, space="PSUM") as psum,
        ):
            # ---------- phase 1: global alpha ----------
            acc = scal.tile([P, KO * NQ], F32)
            for kq in range(KO):
                for nq in range(NQ):
                    w_t = wstage.tile([P, qw], F32, tag="wst", bufs=3,
                                      name=f"wr_{kq}_{nq}")
                    nc.sync.dma_start(
                        out=w_t, in_=w_view[kq, :, nq * qw:(nq + 1) * qw])
                    nc.vector.tensor_reduce(
                        out=acc[:, kq * NQ + nq: kq * NQ + nq + 1], in_=w_t,
                        axis=mybir.AxisListType.X, op=mybir.AluOpType.add,
                        apply_absolute_value=True)

            s_p = scal.tile([P, 1], F32)
            nc.vector.reduce_sum(out=s_p, in_=acc, axis=mybir.AxisListType.X)
            ones = const.tile([P, 1], F32)
            nc.vector.memset(ones, 1.0)
            ps_s = psum.tile([1, 1], F32, tag="mm0", bufs=2, name="ps_s")
            nc.tensor.matmul(ps_s, lhsT=ones, rhs=s_p, start=True, stop=True)
            s_all = scal.tile([1, 1], F32)
            nc.vector.tensor_copy(out=s_all, in_=ps_s)
            nc.sync.dma_start(out=cc_in.ap(), in_=s_all)
            nc.gpsimd.collective_compute(
                "AllReduce", mybir.AluOpType.add,
                replica_groups=[list(range(n_cores))],
                ins=[cc_in.ap()], outs=[cc_out.ap()])

            s_b = scal.tile([P, 1], F32)
            nc.sync.dma_start(out=s_b, in_=cc_out.ap().to_broadcast([P, 1]))
            alpha = scal.tile([P, 1], F32)
            nc.vector.tensor_scalar_mul(alpha, s_b, MEAN_SCALE)
            aeps = scal.tile([P, 1], F32)
            nc.vector.tensor_scalar_add(aeps, alpha, EPS)
            r = scal.tile([P, 1], F32)
            nc.vector.reciprocal(r, aeps)

            # ---------- phase 2: quantize w -> t (bf16 ternary) ----------
            t = tpool.tile([P, KO, N], BF16)
            for kq in range(KO):
                for nq in range(NQ):
                    w_t = wstage.tile([P, qw], F32, tag="wst", bufs=3,
                                      name=f"wq_{kq}_{nq}")
                    nc.sync.dma_start(
                        out=w_t, in_=w_view[kq, :, nq * qw:(nq + 1) * qw])
                    # alternate DVE / GpSimd to double quant throughput
                    eng = nc.vector if (kq * NQ + nq) % 2 == 0 else nc.gpsimd
                    tq = tmpp.tile([P, qw], F32, tag="tq", bufs=2,
                                   name=f"tq_{kq}_{nq}")
                    eng.tensor_scalar(out=tq, in0=w_t, scalar1=r[:, :],
                                      scalar2=MAGIC,
                                      op0=mybir.AluOpType.mult,
                                      op1=mybir.AluOpType.add)
                    tq2 = tmpp.tile([P, qw], F32, tag="tq2", bufs=2,
                                    name=f"tq2_{kq}_{nq}")
                    eng.tensor_scalar(out=tq2, in0=tq, scalar1=MAGIC,
                                      scalar2=1.0,
                                      op0=mybir.AluOpType.subtract,
                                      op1=mybir.AluOpType.min)
                    eng.tensor_scalar(out=t[:, kq, nq * qw:(nq + 1) * qw],
                                      in0=tq2, scalar1=-1.0, scalar2=None,
                                      op0=mybir.AluOpType.max)

            # ---------- phase 3: y = (x @ t^T) * alpha ----------
            for mch in range(MCH):
                xt_t = xstage.tile([P, KO, mc], BF16, tag="xt", bufs=2,
                                   name=f"xt_{mch}")
                nc.sync.dma_start(
                    out=xt_t, in_=xt_view[:, :, mch * mc:(mch + 1) * mc])
                for ms in range(MS):
                    m_idx = mch * MS + ms
                    psums = [
                        psum.tile([P, 512], F32, tag=f"mm{n}", bufs=2,
                                  name=f"ps_{m_idx}_{n}")
                        for n in range(NT)
                    ]
                    for kq in range(KO):
                        lhs = xt_t[:, kq, ms * P:(ms + 1) * P]
                        for n in range(NT):
                            nc.tensor.matmul(
                                psums[n], lhsT=lhs,
                                rhs=t[:, kq, n * 512:(n + 1) * 512],
                                start=(kq == 0), stop=(kq == KO - 1))
                    out_t = outp.tile([P, N], F32, tag="out", bufs=2,
                                      name=f"out_{m_idx}")
                    for n in range(NT):
                        nc.vector.tensor_scalar_mul(
                            out_t[:, n * 512:(n + 1) * 512], psums[n],
                            alpha[:, :])
                    nc.sync.dma_start(out=y_view[m_idx], in_=out_t)

    nc.compile()
    return nc


def kernel(x: np.ndarray, weight: np.ndarray) -> np.ndarray:
    assert x.shape == (B, S, K) and weight.shape == (N_FULL, K)

    # host-side layout prep (no math beyond the bf16 cast of x)
    xt = np.ascontiguousarray(
        x.reshape(M, K).astype(ml_dtypes.bfloat16).T)          # [K, M] bf16
    in_maps = []
    for c in range(N_CORES):
        wt_c = np.ascontiguousarray(
            weight[c * N:(c + 1) * N, :].T.astype(np.float32))  # [K, N] f32
        in_maps.append({"xt": xt, "wt": wt_c})

    nc = build_nc(M, K, N)
    res = run_bass_kernel_spmd(nc, in_maps, list(range(N_CORES)))
    y = np.concatenate([res.results[c]["y"] for c in range(N_CORES)], axis=1)
    return np.ascontiguousarray(y.reshape(B, S, N_FULL).astype(np.float32))


# revision 12
# speedup vs baseline: 1.3678x; 1.3678x over previous
"""BitLinear (ternary-quantized linear) Trainium2 kernel.

Computes y = x @ w_q^T where w_q = clip(round(w/(alpha+eps)), -1, 1) * alpha
and alpha = mean(|w|) over the FULL weight.

Distribution (8 NeuronCores, tensor-parallel):
  - weight rows (out_features) sharded 8 ways: each core owns N = 16384/8 = 2048
    output features.
  - x replicated to every core (pre-transposed + bf16 on host for layout).
  - alpha needs the global |w| mean: each core reduces its own shard, then an
    on-device AllReduce across the 8 cores produces the global sum.
  - outputs stay column-sharded; host concatenates the 8 shards.

Per-core device kernel:
  phase 1: abs-sum reduce of w shard (DVE reduce w/ apply_absolute_value),
           cross-partition sum via PE ones-matmul, AllReduce, then
           alpha = S * 2^-26, r = 1/(alpha+eps) (DVE reciprocal).
  phase 2: quantize w -> t in {-1,0,+1} as bf16 (exact), using the fp32
           round-to-nearest-even "magic number" trick: RNE(z) = (z+1.5*2^23)-1.5*2^23.
           t kept entirely in SBUF ([128, 32, 2048] bf16 = 128KB/partition).
  phase 3: y = x @ t^T on the PE in bf16 (fp32 PSUM accumulation), tiled
           [128m x 128k x 512n]; PSUM evicted with a fused *alpha scale on DVE.
"""

import numpy as np
import ml_dtypes

import concourse.bass as bass
import concourse.mybir as mybir
import concourse.tile as tile
from concourse import bacc
from concourse.bass_utils import run_bass_kernel_spmd

P = 128
N_CORES = 8

# Full problem shapes (hardcoded per contract).
B, S, K = 4, 2048, 4096
N_FULL = 16384
M = B * S                      # 8192 rows of x
N = N_FULL // N_CORES          # 2048 output features per core

MAGIC = 12582912.0             # 1.5 * 2**23: fp32 RNE rounding constant
EPS = 1e-8
MEAN_SCALE = float(2.0 ** -26)  # 1 / (16384*4096), exact power of two

F32 = mybir.dt.float32
BF16 = mybir.dt.bfloat16


def build_nc(M, K, N, n_cores=N_CORES, mc=128, qw=2048):
    """Build the per-core Bass program. All cores run the same program (SPMD)."""
    KO = K // P          # k tiles of 128 partitions
    NT = N // 512        # matmul n tiles (psum banks per m-subtile)
    NQ = max(N // qw, 1) # quantization column tiles per k tile
    qw = min(qw, N)
    MCH = M // mc        # m chunks
    MS = mc // P         # m subtiles per chunk

    nc = bacc.Bacc("TRN2", target_bir_lowering=False, debug=False,
                   num_devices=n_cores)

    # xt is pre-tiled on host: [MCH, 128, KO*mc] so each m-chunk is one fully
    # contiguous DMA (16KB per partition row).
    xt = nc.dram_tensor("xt", [MCH, P, KO * mc], BF16, kind="ExternalInput")
    wt = nc.dram_tensor("wt", [K, N], F32, kind="ExternalInput")
    y = nc.dram_tensor("y", [M, N], F32, kind="ExternalOutput")
    cc_in = nc.dram_tensor("cc_in", [1, 1], F32)
    cc_out = nc.dram_tensor("cc_out", [1, 1], F32, addr_space="Shared")

    w_view = wt.ap().rearrange("(ko p) n -> ko p n", p=P)    # [KO, 128, N]
    y_view = y.ap().rearrange("(mo p) n -> mo p n", p=P)     # [M/128, 128, N]

    with tile.TileContext(nc) as tc:
        with (
            tc.tile_pool(name="const", bufs=1) as const,
            tc.tile_pool(name="scal", bufs=1) as scal,
            tc.tile_pool(name="wstage", bufs=3) as wstage,
            tc.tile_pool(name="tmp", bufs=2) as tmpp,
            tc.tile_pool(name="tpool", bufs=1) as tpool,
            tc.tile_pool(name="xstage", bufs=2) as xstage,
            tc.tile_pool(name="outp", bufs=2) as outp,
            tc.tile_pool(name="psum", bufs=1, space="PSUM") as psum,
        ):
            # ---------- phase 1: global alpha ----------
            acc = scal.tile([P, KO * NQ], F32)
            for kq in range(KO):
                for nq in range(NQ):
                    w_t = wstage.tile([P, qw], F32, tag="wst", bufs=2,
                                      name=f"wr_{kq}_{nq}")
                    nc.sync.dma_start(
                        out=w_t, in_=w_view[kq, :, nq * qw:(nq + 1) * qw])
                    nc.vector.tensor_reduce(
                        out=acc[:, kq * NQ + nq: kq * NQ + nq + 1], in_=w_t,
                        axis=mybir.AxisListType.X, op=mybir.AluOpType.add,
                        apply_absolute_value=True)

            s_p = scal.tile([P, 1], F32)
            nc.vector.reduce_sum(out=s_p, in_=acc, axis=mybir.AxisListType.X)
            ones = const.tile([P, 1], F32)
            nc.vector.memset(ones, 1.0)
            ps_s = psum.tile([1, 1], F32, tag="mm0", bufs=2, name="ps_s")
            nc.tensor.matmul(ps_s, lhsT=ones, rhs=s_p, start=True, stop=True)
            s_all = scal.tile([1, 1], F32)
            nc.vector.tensor_copy(out=s_all, in_=ps_s)
            nc.sync.dma_start(out=cc_in.ap(), in_=s_all)
            nc.gpsimd.collective_compute(
                "AllReduce", mybir.AluOpType.add,
                replica_groups=[list(range(n_cores))],
                ins=[cc_in.ap()], outs=[cc_out.ap()])

            s_b = scal.tile([P, 1], F32)
            nc.sync.dma_start(out=s_b, in_=cc_out.ap().to_broadcast([P, 1]))
            alpha = scal.tile([P, 1], F32)
            nc.vector.tensor_scalar_mul(alpha, s_b, MEAN_SCALE)
            aeps = scal.tile([P, 1], F32)
            nc.vector.tensor_scalar_add(aeps, alpha, EPS)
            r = scal.tile([P, 1], F32)
            nc.vector.reciprocal(r, aeps)

            # ---------- phase 2: quantize w -> t (bf16 ternary) ----------
            # DVE: z = RNE(w*r) + MAGIC (dual op mult+add), then clamp in
            # magic space to [MAGIC-1, MAGIC+1] (dual op min+max).
            # ACT: subtract MAGIC and cast to bf16 (Copy with bias=-MAGIC).
            t = tpool.tile([P, KO, N], BF16)
            for kq in range(KO):
                for nq in range(NQ):
                    w_t = wstage.tile([P, qw], F32, tag="wst", bufs=2,
                                      name=f"wq_{kq}_{nq}")
                    nc.sync.dma_start(
                        out=w_t, in_=w_view[kq, :, nq * qw:(nq + 1) * qw])
                    tq = tmpp.tile([P, qw], F32, tag="tq", bufs=2,
                                   name=f"tq_{kq}_{nq}")
                    nc.vector.tensor_scalar(out=tq, in0=w_t, scalar1=r[:, :],
                                            scalar2=MAGIC,
                                            op0=mybir.AluOpType.mult,
                                            op1=mybir.AluOpType.add)
                    nc.vector.tensor_scalar(out=tq, in0=tq,
                                            scalar1=MAGIC + 1.0,
                                            scalar2=MAGIC - 1.0,
                                            op0=mybir.AluOpType.min,
                                            op1=mybir.AluOpType.max)
                    nc.scalar.activation(
                        out=t[:, kq, nq * qw:(nq + 1) * qw], in_=tq,
                        func=mybir.ActivationFunctionType.Copy,
                        bias=-MAGIC, scale=1.0)

            # ---------- phase 3: y = (x @ t^T) * alpha ----------
            for mch in range(MCH):
                xt_t = xstage.tile([P, KO, mc], BF16, tag="xt", bufs=2,
                                   name=f"xt_{mch}")
                nc.sync.dma_start(
                    out=xt_t,
                    in_=xt.ap()[mch].rearrange("p (ko m) -> p ko m", ko=KO))
                for ms in range(MS):
                    m_idx = mch * MS + ms
                    psums = [
                        psum.tile([P, 512], F32, tag=f"mm{n}", bufs=2,
                                  name=f"ps_{m_idx}_{n}")
                        for n in range(NT)
                    ]
                    for kq in range(KO):
                        lhs = xt_t[:, kq, ms * P:(ms + 1) * P]
                        for n in range(NT):
                            nc.tensor.matmul(
                                psums[n], lhsT=lhs,
                                rhs=t[:, kq, n * 512:(n + 1) * 512],
                                start=(kq == 0), stop=(kq == KO - 1))
                    out_t = outp.tile([P, N], F32, tag="out", bufs=2,
                                      name=f"out_{m_idx}")
                    for n in range(NT):
                        nc.vector.tensor_scalar_mul(
                            out_t[:, n * 512:(n + 1) * 512], psums[n],
                            alpha[:, :])
                    nc.sync.dma_start(out=y_view[m_idx], in_=out_t)

    nc.compile()
    return nc


def prep_xt(x_flat: np.ndarray, mc: int) -> np.ndarray:
    """Pre-tile x for contiguous chunk DMA:
    xt[mch, p, ko*mc+m] = bf16(x_flat[mch*mc+m, ko*128+p])."""
    m, k = x_flat.shape
    mch, ko = m // mc, k // P
    return np.ascontiguousarray(
        x_flat.reshape(mch, mc, ko, P).astype(ml_dtypes.bfloat16)
        .transpose(0, 3, 2, 1)).reshape(mch, P, ko * mc)


def kernel(x: np.ndarray, weight: np.ndarray) -> np.ndarray:
    assert x.shape == (B, S, K) and weight.shape == (N_FULL, K)

    # host-side layout prep (no math beyond the bf16 cast of x)
    xt = prep_xt(x.reshape(M, K), 128)
    in_maps = []
    for c in range(N_CORES):
        wt_c = np.ascontiguousarray(
            weight[c * N:(c + 1) * N, :].T.astype(np.float32))  # [K, N] f32
        in_maps.append({"xt": xt, "wt": wt_c})

    nc = build_nc(M, K, N)
    res = run_bass_kernel_spmd(nc, in_maps, list(range(N_CORES)))
    y = np.concatenate([res.results[c]["y"] for c in range(N_CORES)], axis=1)
    return np.ascontiguousarray(y.reshape(B, S, N_FULL).astype(np.float32))


# revision 14
# speedup vs baseline: 1.4271x; 1.0434x over previous
"""BitLinear (ternary-quantized linear) Trainium2 kernel.

Computes y = x @ w_q^T where w_q = clip(round(w/(alpha+eps)), -1, 1) * alpha
and alpha = mean(|w|) over the FULL weight.

Distribution (8 NeuronCores, tensor-parallel):
  - weight rows (out_features) sharded 8 ways: each core owns N = 16384/8 = 2048
    output features.
  - x replicated to every core (pre-transposed + bf16 on host for layout).
  - alpha needs the global |w| mean: each core reduces its own shard, then an
    on-device AllReduce across the 8 cores produces the global sum.
  - outputs stay column-sharded; host concatenates the 8 shards.

Per-core device kernel:
  phase 1: abs-sum reduce of w shard (DVE reduce w/ apply_absolute_value),
           cross-partition sum via PE ones-matmul, AllReduce, then
           alpha = S * 2^-26, r = 1/(alpha+eps) (DVE reciprocal).
  phase 2: quantize w -> t in {-1,0,+1} as bf16 (exact), using the fp32
           round-to-nearest-even "magic number" trick: RNE(z) = (z+1.5*2^23)-1.5*2^23.
           t kept entirely in SBUF ([128, 32, 2048] bf16 = 128KB/partition).
  phase 3: y = x @ t^T on the PE in bf16 (fp32 PSUM accumulation), tiled
           [128m x 128k x 512n]; PSUM evicted with a fused *alpha scale on DVE.
"""

import numpy as np
import ml_dtypes

import concourse.bass as bass
import concourse.mybir as mybir
import concourse.tile as tile
from concourse import bacc
from concourse.bass_utils import run_bass_kernel_spmd

P = 128
N_CORES = 8

# Full problem shapes (hardcoded per contract).
B, S, K = 4, 2048, 4096
N_FULL = 16384
M = B * S                      # 8192 rows of x
N = N_FULL // N_CORES          # 2048 output features per core

MAGIC = 12582912.0             # 1.5 * 2**23: fp32 RNE rounding constant
EPS = 1e-8
MEAN_SCALE = float(2.0 ** -26)  # 1 / (16384*4096), exact power of two

F32 = mybir.dt.float32
BF16 = mybir.dt.bfloat16


def build_nc(M, K, N, n_cores=N_CORES, mc=128, qw=2048):
    """Build the per-core Bass program. All cores run the same program (SPMD)."""
    KO = K // P          # k tiles of 128 partitions
    NT = N // 512        # matmul n tiles (psum banks per m-subtile)
    NQ = max(N // qw, 1) # quantization column tiles per k tile
    qw = min(qw, N)
    MCH = M // mc        # m chunks
    MS = mc // P         # m subtiles per chunk

    nc = bacc.Bacc("TRN2", target_bir_lowering=False, debug=False,
                   num_devices=n_cores)

    # xt is pre-tiled on host: [MCH, 128, KO*mc] so each m-chunk is one fully
    # contiguous DMA (16KB per partition row).
    xt = nc.dram_tensor("xt", [MCH, P, KO * mc], BF16, kind="ExternalInput")
    wt = nc.dram_tensor("wt", [K, N], F32, kind="ExternalInput")
    y = nc.dram_tensor("y", [M, N], F32, kind="ExternalOutput")
    cc_in = nc.dram_tensor("cc_in", [1, 1], F32)
    cc_out = nc.dram_tensor("cc_out", [1, 1], F32, addr_space="Shared")

    w_view = wt.ap().rearrange("(ko p) n -> ko p n", p=P)    # [KO, 128, N]
    y_view = y.ap().rearrange("(mo p) n -> mo p n", p=P)     # [M/128, 128, N]

    with tile.TileContext(nc) as tc:
        with (
            tc.tile_pool(name="const", bufs=1) as const,
            tc.tile_pool(name="scal", bufs=1) as scal,
            tc.tile_pool(name="wstage", bufs=3) as wstage,
            tc.tile_pool(name="tmp", bufs=2) as tmpp,
            tc.tile_pool(name="tpool", bufs=1) as tpool,
            tc.tile_pool(name="xstage", bufs=2) as xstage,
            tc.tile_pool(name="outp", bufs=2) as outp,
            tc.tile_pool(name="psum", bufs=1, space="PSUM") as psum,
        ):
            # ---------- phase 1: global alpha ----------
            # |w| row-sums split between DVE (tensor_reduce w/ abs) and ACT
            # (activation Abs with accum_out) so the reduce tail is half as long.
            acc = scal.tile([P, KO * NQ], F32)
            for kq in range(KO):
                for nq in range(NQ):
                    idx = kq * NQ + nq
                    w_t = wstage.tile([P, qw], F32, tag="wst", bufs=2,
                                      name=f"wr_{kq}_{nq}")
                    nc.sync.dma_start(
                        out=w_t, in_=w_view[kq, :, nq * qw:(nq + 1) * qw])
                    if idx % 2 == 0:
                        nc.vector.tensor_reduce(
                            out=acc[:, idx:idx + 1], in_=w_t,
                            axis=mybir.AxisListType.X, op=mybir.AluOpType.add,
                            apply_absolute_value=True)
                    else:
                        scratch = tmpp.tile([P, qw], F32, tag="tq", bufs=2,
                                            name=f"sc_{kq}_{nq}")
                        nc.scalar.activation(
                            out=scratch, in_=w_t,
                            func=mybir.ActivationFunctionType.Abs,
                            accum_out=acc[:, idx:idx + 1])

            s_p = scal.tile([P, 1], F32)
            nc.vector.reduce_sum(out=s_p, in_=acc, axis=mybir.AxisListType.X)
            ones = const.tile([P, 1], F32)
            nc.vector.memset(ones, 1.0)
            ps_s = psum.tile([1, 1], F32, tag="mm0", bufs=2, name="ps_s")
            nc.tensor.matmul(ps_s, lhsT=ones, rhs=s_p, start=True, stop=True)
            s_all = scal.tile([1, 1], F32)
            nc.vector.tensor_copy(out=s_all, in_=ps_s)
            nc.sync.dma_start(out=cc_in.ap(), in_=s_all)
            nc.gpsimd.collective_compute(
                "AllReduce", mybir.AluOpType.add,
                replica_groups=[list(range(n_cores))],
                ins=[cc_in.ap()], outs=[cc_out.ap()])

            s_b = scal.tile([P, 1], F32)
            nc.sync.dma_start(out=s_b, in_=cc_out.ap().to_broadcast([P, 1]))
            alpha = scal.tile([P, 1], F32)
            nc.vector.tensor_scalar_mul(alpha, s_b, MEAN_SCALE)
            aeps = scal.tile([P, 1], F32)
            nc.vector.tensor_scalar_add(aeps, alpha, EPS)
            r = scal.tile([P, 1], F32)
            nc.vector.reciprocal(r, aeps)

            # ---------- phase 2: quantize w -> t (bf16 ternary) ----------
            # DVE: z = RNE(w*r) + MAGIC (dual op mult+add), then clamp in
            # magic space to [MAGIC-1, MAGIC+1] (dual op min+max).
            # ACT: subtract MAGIC and cast to bf16 (Copy with bias=-MAGIC).
            t = tpool.tile([P, KO, N], BF16)
            for kq in range(KO):
                for nq in range(NQ):
                    w_t = wstage.tile([P, qw], F32, tag="wst", bufs=2,
                                      name=f"wq_{kq}_{nq}")
                    nc.sync.dma_start(
                        out=w_t, in_=w_view[kq, :, nq * qw:(nq + 1) * qw])
                    tq = tmpp.tile([P, qw], F32, tag="tq", bufs=2,
                                   name=f"tq_{kq}_{nq}")
                    if (kq * NQ + nq) % 2 == 0:
                        nc.vector.tensor_scalar(out=tq, in0=w_t,
                                                scalar1=r[:, :],
                                                scalar2=MAGIC,
                                                op0=mybir.AluOpType.mult,
                                                op1=mybir.AluOpType.add)
                    else:
                        nc.scalar.activation(
                            out=tq, in_=w_t,
                            func=mybir.ActivationFunctionType.Copy,
                            bias=MAGIC, scale=r[:, :])
                    nc.vector.tensor_scalar(out=tq, in0=tq,
                                            scalar1=MAGIC + 1.0,
                                            scalar2=MAGIC - 1.0,
                                            op0=mybir.AluOpType.min,
                                            op1=mybir.AluOpType.max)
                    nc.scalar.activation(
                        out=t[:, kq, nq * qw:(nq + 1) * qw], in_=tq,
                        func=mybir.ActivationFunctionType.Copy,
                        bias=-MAGIC, scale=1.0)

            # ---------- phase 3: y = (x @ t^T) * alpha ----------
            for mch in range(MCH):
                xt_t = xstage.tile([P, KO, mc], BF16, tag="xt", bufs=2,
                                   name=f"xt_{mch}")
                nc.sync.dma_start(
                    out=xt_t,
                    in_=xt.ap()[mch].rearrange("p (ko m) -> p ko m", ko=KO))
                for ms in range(MS):
                    m_idx = mch * MS + ms
                    psums = [
                        psum.tile([P, 512], F32, tag=f"mm{n}", bufs=2,
                                  name=f"ps_{m_idx}_{n}")
                        for n in range(NT)
                    ]
                    for kq in range(KO):
                        lhs = xt_t[:, kq, ms * P:(ms + 1) * P]
                        for n in range(NT):
                            nc.tensor.matmul(
                                psums[n], lhsT=lhs,
                                rhs=t[:, kq, n * 512:(n + 1) * 512],
                                start=(kq == 0), stop=(kq == KO - 1))
                    out_t = outp.tile([P, N], F32, tag="out", bufs=2,
                                      name=f"out_{m_idx}")
                    for n in range(NT):
                        nc.vector.tensor_scalar_mul(
                            out_t[:, n * 512:(n + 1) * 512], psums[n],
                            alpha[:, :])
                    nc.sync.dma_start(out=y_view[m_idx], in_=out_t)

    nc.compile()
    return nc


def prep_xt(x_flat: np.ndarray, mc: int) -> np.ndarray:
    """Pre-tile x for contiguous chunk DMA:
    xt[mch, p, ko*mc+m] = bf16(x_flat[mch*mc+m, ko*128+p])."""
    m, k = x_flat.shape
    mch, ko = m // mc, k // P
    return np.ascontiguousarray(
        x_flat.reshape(mch, mc, ko, P).astype(ml_dtypes.bfloat16)
        .transpose(0, 3, 2, 1)).reshape(mch, P, ko * mc)


def kernel(x: np.ndarray, weight: np.ndarray) -> np.ndarray:
    assert x.shape == (B, S, K) and weight.shape == (N_FULL, K)

    # host-side layout prep (no math beyond the bf16 cast of x)
    xt = prep_xt(x.reshape(M, K), 128)
    in_maps = []
    for c in range(N_CORES):
        wt_c = np.ascontiguousarray(
            weight[c * N:(c + 1) * N, :].T.astype(np.float32))  # [K, N] f32
        in_maps.append({"xt": xt, "wt": wt_c})

    nc = build_nc(M, K, N)
    res = run_bass_kernel_spmd(nc, in_maps, list(range(N_CORES)))
    y = np.concatenate([res.results[c]["y"] for c in range(N_CORES)], axis=1)
    return np.ascontiguousarray(y.reshape(B, S, N_FULL).astype(np.float32))
